# revision 36
# baseline (speedup 1.0000x reference)
"""Bi-attention kernel for Trainium2 (Bass/Tile), 8-core data-parallel over batch.

Problem (per batch element b, full shapes x:[8,2048,1024] f32, mask:[8,2048] i32):
    score   = x_b @ x_b.T                      [2048, 2048]
    score   = where(mask==0, -inf, score)      (mask keys)
    attn    = softmax(score, axis=-1)
    context = attn @ x_b                       [2048, 1024]
    out_b   = concat([x, ctx, x+ctx, x-ctx, x*ctx], -1)   [2048, 5120]

Sharding: batch dim (8) across the 8 NeuronCores, one batch element per core.
No cross-core communication.

Per-core schedule (S=2048, D=1024, P=128):
  setup: stream x in 16 row-chunks (halved DMAs); PE-transpose each (batched
         4-wide through one PSUM bank) into 4 key-group tiles xTg[g]
         (float32r, d on partitions) so the first score matmuls can start
         after ~2MB of load; cast a resident fp16 natural-layout copy for the
         context matmul; build the additive key-mask row (-1e5 on masked
         keys) with a small int8 casting broadcast DMA so it doesn't stall
         the serial x-load stream.
  per q-tile (16 x 128 queries), software-pipelined one tile ahead:
    scores: 4 key-chunks of 512, each accumulating 8 float32r matmuls
            (d contracted) into a PSUM bank; a tensor_add drains PSUM + key
            mask into SBUF and a per-chunk reduce_max feeds the row max.
    softmax: ACT exp per 1024-half, bias=-rowmax, fp16 out, denominators via
            accum_out (masked keys underflow to exactly 0); halving lets the
            first p-transposes start before the second exp finishes.
    context: PE-transposes p in 2 batches of 8 through one PSUM bank (fp16
            [128,1024] = 2KB = one bank), one [128,1024] copy per batch
            (DVE/ACT alternating); 2x16 fp16 matmuls into [128,512] PSUM
            tiles, each drained by an ACT copy scaled with 1/denom straight
            into the output tile.
    output: x DMA'd into cols [0,1024) and copied on to out block 0;
            +,-,* on Pool/DVE per 512-half; per-block-half DMAs out so the
            tail flush after the final matmul is short.

float32r (TF32-like, ~1.5e-4 rel err, 1 cyc/row at N=512 vs 4 for fp32) covers
the score matmul: softmax weights see <=~2% worst-case perturbation on
near-tied keys, well below tolerance; fp16 suffices for the convex-combination
context matmul. PE is the bottleneck engine (~250us of matmul+transpose work).
"""

import os

os.environ.setdefault("JAX_PLATFORMS", "axon")  # NEFF executes via the axon PJRT tunnel

import numpy as np

import concourse.bass as bass
import concourse.tile as tile
from concourse import bacc, mybir
from concourse.bass_utils import run_bass_kernel_spmd
from concourse.masks import make_identity

P = 128
S = 2048
D = 1024
NQ = S // P          # 16 q tiles
KD = D // P          # 8 d subtiles (score contraction)
NG = 4               # xT key groups of 512
NB = 8               # batch / cores
DT = mybir.dt
MASK_NEG = -1.0e5


def _build():
    nc = bacc.Bacc()
    x = nc.dram_tensor("x", (S, D), DT.float32, kind="ExternalInput")
    mask = nc.dram_tensor("mask", (S,), DT.int32, kind="ExternalInput")
    out = nc.dram_tensor("out", (S, 5 * D), DT.float32, kind="ExternalOutput")

    with tile.TileContext(nc) as tc:
        with (
            tc.tile_pool(name="const", bufs=1) as const,
            tc.tile_pool(name="ps_s", bufs=4, space="PSUM") as ps_s,
            tc.tile_pool(name="ps_t", bufs=2, space="PSUM") as ps_t,
            tc.tile_pool(name="ps_c", bufs=2, space="PSUM") as ps_c,
        ):
            ident = const.tile([P, P], DT.float32)
            make_identity(nc, ident)
            ident_bf = const.tile([P, P], DT.float16)
            nc.vector.tensor_copy(ident_bf[:], ident[:])

            # resident operands
            xTg = [
                const.tile([P, KD, 512], DT.float32r, name=f"xTg{g}")
                for g in range(NG)
            ]
            xnb = const.tile([P, NQ, D], DT.float16)    # x natural, fp16
            maskb = const.tile([P, S], DT.float32)      # additive key mask

            with tc.tile_pool(name="setup", bufs=3) as setup, \
                 tc.tile_pool(name="xin_pool", bufs=6) as xin_pool:
                # stream x; PE-transpose into xTg (f32r) 4-wide per PSUM bank;
                # bf16 natural copy for the context matmul. x loads are split
                # in halves so the first transposes start ~1us in.
                for ci in range(NQ):
                    xin = xin_pool.tile([P, D], DT.float32, tag="xin")
                    nc.sync.dma_start(xin[:, 0:512], x[ci * P:(ci + 1) * P, 0:512])
                    nc.sync.dma_start(xin[:, 512:D], x[ci * P:(ci + 1) * P, 512:D])
                    nc.vector.tensor_copy(xnb[:, ci, :], xin[:])
                    if ci == 0:
                        # additive key mask, broadcast across partitions:
                        # (mask - 1) * 1e5 -> 0 keep, -1e5 masked. Emitted after
                        # the first x chunk so it doesn't gate the PE pipeline.
                        mask_ap = mask[:]
                        mask_i8 = setup.tile([P, S], DT.int8, tag="mask_i8")
                        nc.gpsimd.dma_start(   # casting broadcast: 256KB not 1MB
                            out=mask_i8[:],
                            in_=bass.AP(
                                tensor=mask_ap.tensor,
                                offset=mask_ap.offset,
                                ap=[[0, P], mask_ap.ap[0]],
                            ),
                        )
                        nc.vector.tensor_scalar(
                            out=maskb[:],
                            in0=mask_i8[:],
                            scalar1=1.0,
                            scalar2=-MASK_NEG,
                            op0=mybir.AluOpType.subtract,
                            op1=mybir.AluOpType.mult,
                        )
                    g, col = ci // 4, (ci % 4) * P
                    for jb in range(2):           # batches of 4 d-subtiles
                        pst = ps_t.tile([P, 4 * P], DT.float32, tag="pst")
                        for j4 in range(4):
                            j = jb * 4 + j4
                            nc.tensor.transpose(
                                pst[:, j4 * P:(j4 + 1) * P],
                                xin[:, j * P:(j + 1) * P],
                                ident[:],
                            )
                        dst = xTg[g][:, jb * 4:(jb + 1) * 4, col:col + P]
                        src = pst[:].rearrange("p (j q) -> p j q", j=4)
                        if (ci + jb) % 2 == 0:
                            nc.vector.tensor_copy(dst, src)
                        else:
                            nc.scalar.copy(dst, src)

            with tc.tile_pool(name="work", bufs=2) as work, \
                 tc.tile_pool(name="pwork", bufs=3) as pwork, \
                 tc.tile_pool(name="stats", bufs=4) as stats:
                def emit_scores(qi):
                    """scores (f32r) + mask + row-max, half-rows of 1024."""
                    q_sl = slice(qi * P, (qi + 1) * P)
                    qg, qcol = qi // 4, (qi % 4) * P
                    s_sb = work.tile([P, S], DT.float32, tag="s_sb", name=f"s_sb{qi}")
                    rm = stats.tile([P, NG], DT.float32, tag="rm", name=f"rm{qi}")
                    for g in range(NG):
                        pss = ps_s.tile([P, 512], DT.float32, tag="pss", name=f"pss{qi}_{g}")
                        for j in range(KD):
                            nc.tensor.matmul(
                                pss[:],
                                xTg[qg][:, j, qcol:qcol + P],
                                xTg[g][:, j, :],
                                start=(j == 0),
                                stop=(j == KD - 1),
                            )
                        nc.vector.tensor_add(
                            s_sb[:, g * 512:(g + 1) * 512],
                            pss[:],
                            maskb[:, g * 512:(g + 1) * 512],
                        )
                        nc.vector.reduce_max(
                            rm[:, g:g + 1],
                            s_sb[:, g * 512:(g + 1) * 512],
                            axis=mybir.AxisListType.X,
                        )
                    return s_sb, rm

                def emit_rest(qi, s_sb, rm, nchunk=2, fa=1):
                    """softmax, p-transpose, context, output assembly + DMA."""
                    q_sl = slice(qi * P, (qi + 1) * P)
                    m = stats.tile([P, 1], DT.float32, tag="m", name=f"m{qi}")
                    nc.vector.reduce_max(m[:], rm[:], axis=mybir.AxisListType.X)
                    negm = stats.tile([P, 1], DT.float32, tag="negm", name=f"negm{qi}")
                    nc.vector.tensor_scalar_mul(negm[:], m[:], -1.0)

                    # exp per 1024-half: downstream transposes/ctx matmuls on
                    # the first half start ~1us earlier
                    p_bf = pwork.tile([P, S], DT.float16, tag="p_bf", name=f"p_bf{qi}")
                    dsum = stats.tile([P, 2], DT.float32, tag="dsum", name=f"dsum{qi}")
                    for h in range(2):
                        nc.scalar.activation(
                            out=p_bf[:, h * 1024:(h + 1) * 1024],
                            in_=s_sb[:, h * 1024:(h + 1) * 1024],
                            func=mybir.ActivationFunctionType.Exp,
                            bias=negm[:],
                            scale=1.0,
                            accum_out=dsum[:, h:h + 1],
                        )
                    denom = stats.tile([P, 1], DT.float32, tag="denom", name=f"denom{qi}")
                    nc.vector.reduce_sum(denom[:], dsum[:], axis=mybir.AxisListType.X)
                    recip = stats.tile([P, 1], DT.float32, tag="recip", name=f"recip{qi}")
                    nc.vector.reciprocal(recip[:], denom[:])

                    # transpose p, 2 batches of 8 through one PSUM bank
                    # (fp16 [128,1024] = 2KB = one bank; fewer batch
                    # boundaries and half the PSUM-drain copies)
                    pT = pwork.tile([P, S], DT.float16, tag="pT", name=f"pT{qi}")
                    for b in range(2):
                        pst = ps_t.tile([P, 8 * P], DT.float16, tag="pst", name=f"pstp{qi}_{b}")
                        for t8 in range(8):
                            t = b * 8 + t8
                            nc.tensor.transpose(
                                pst[:, t8 * P:(t8 + 1) * P],
                                p_bf[:, t * P:(t + 1) * P],
                                ident_bf[:],
                            )
                        dst = pT[:, b * 8 * P:(b + 1) * 8 * P]
                        if b % 2 == 0:
                            nc.vector.tensor_copy(dst, pst[:])
                        else:
                            nc.scalar.copy(dst, pst[:])

                    # output tile
                    o_sb = work.tile([P, 5 * D], DT.float32, tag="o_sb", name=f"o_sb{qi}")
                    nc.sync.dma_start(o_sb[:, 0:D], x[q_sl, :])
                    nc.sync.dma_start(out[q_sl, 0:D], o_sb[:, 0:D])

                    # context (fp16); drain + assemble + store per chunk so the
                    # flush after the final matmul is short (the last q-tile
                    # uses 4x256 chunks to halve the tail chain)
                    W = D // nchunk
                    for dc in range(nchunk):
                        psc = ps_c.tile([P, 512], DT.float32, tag="psc", name=f"psc{qi}_{dc}")
                        for t in range(NQ):
                            nc.tensor.matmul(
                                psc[:, :W],
                                pT[:, t * P:(t + 1) * P],
                                xnb[:, t, dc * W:(dc + 1) * W],
                                start=(t == 0),
                                stop=(t == NQ - 1),
                            )
                        FW = W // fa
                        for f in range(fa):
                            lo = dc * W + f * FW
                            xh = o_sb[:, lo:lo + FW]
                            ch = o_sb[:, D + lo:D + lo + FW]
                            nc.scalar.mul(ch, psc[:, f * FW:(f + 1) * FW], recip[:])
                            nc.gpsimd.tensor_add(
                                o_sb[:, 2 * D + lo:2 * D + lo + FW], xh, ch
                            )
                            nc.vector.tensor_sub(
                                o_sb[:, 3 * D + lo:3 * D + lo + FW], xh, ch
                            )
                            nc.vector.tensor_mul(
                                o_sb[:, 4 * D + lo:4 * D + lo + FW], xh, ch
                            )
                            for blk in range(1, 5):
                                nc.sync.dma_start(
                                    out[q_sl, blk * D + lo:blk * D + lo + FW],
                                    o_sb[:, blk * D + lo:blk * D + lo + FW],
                                )

                # 2-stage software pipeline: scores run one q-tile ahead so the
                # softmax/transpose latency of tile qi hides under the score
                # matmuls of tile qi+1.
                pending = emit_scores(0)
                for qi in range(1, NQ):
                    nxt = emit_scores(qi)
                    emit_rest(qi - 1, *pending)
                    pending = nxt
                emit_rest(NQ - 1, *pending)

    nc.finalize()
    return nc


_NC_CACHE = None


def _get_nc():
    global _NC_CACHE
    if _NC_CACHE is None:
        _NC_CACHE = _build()
    return _NC_CACHE


def kernel(x, mask, _trace=False):
    x = np.asarray(x, dtype=np.float32)
    mask = np.asarray(mask, dtype=np.int32)
    assert x.shape == (NB, S, D), x.shape
    assert mask.shape == (NB, S), mask.shape

    nc = _get_nc()
    in_maps = [
        {"x": np.ascontiguousarray(x[b]), "mask": np.ascontiguousarray(mask[b])}
        for b in range(NB)
    ]
    res = run_bass_kernel_spmd(nc, in_maps, core_ids=list(range(NB)), trace=_trace)
    out = np.stack([r["out"] for r in res.results], axis=0)
    if _trace:
        return out, res
    return out
